# revision 25
# baseline (speedup 1.0000x reference)
"""Trainium2 Bass kernel for nn_DifferentialGCNBlock (intra-spatial GCN + inter-frame GCN).

Sharding: 8 cores = 4 batches x 2 node-halves. Each core computes both GCN stages
fully locally for its (batch, 512-node half), using a 64-node halo on each side
(A_sp is a 3x3x3 stencil => bandwidth |i-j| <= 73 < 128).

Math per core (c-major activations matching DRAM layout, all matmul data fp16,
PSUM accumulation fp32):
  x^T[f] (C=256 part, M=640 free)  --DMA (fp16, 5 frames/transfer)-->  SBUF
  P[f]   = x W_intra          : lhsT = x^T slices (c,m), rhs = Wi  -> P (m part, c')
  T^T[f] = (A_sp P)^T         : lhsT = P slices (m,c'), rhs = A band blocks (m,n)
  yh[f]  = relu(dinv[f]*T^T)  : ACT, PSUM->SBUF fp16 (c' part, n)
  Zs[f'] = Wo^T (yh[f'-1]+yh[f']+yh[f'+1])   : 3-frame sum folded into PSUM
                                               accumulation (6 matmuls)
  out[f'] = relu(dinv[f']*Zs[f'])            : ACT, PSUM->SBUF fp32, paired DMA out
The inter-frame path graph's normalized adjacency is separable: A_fr[f',f] =
dinv[f']*dinv[f] for |f-f'|<=1, which is what the yh scaling + output scaling use.
"""
import sys

for p in ("/opt/trn_rl_repo",):
    if p not in sys.path:
        sys.path.insert(0, p)

import numpy as np

H, W_, D = 16, 8, 8
N = H * W_ * D          # 1024
F = 25
C = 256
BS = 4
HALO = 64
NLOC = 512
M = HALO + NLOC + HALO  # 640
NCORES = 8
FPL = 5                 # frames per input DMA (25 = 5*5)
_XCHUNKS = [1, 2, 3, 3, 4, 4, 4, 4]  # small first chunks so compute starts early

# banded structure of A_sp in local coords: m = n + 64 + delta, |delta| <= 73
# k-tile j (m in [128j, 128j+128)) touches n in [128j-137, 128j+137)
_BANDS = [(0, 144), (0, 272), (112, 400), (240, 512), (368, 512)]
# order: j=2's band [112,400) overlaps every other band, so putting it first
# (start=True, widened to the full bank) guarantees the whole-bank has_written
# clear happens first; the remaining matmuls accumulate per-element and are
# order-independent.
_BAND_ORDER = [2, 0, 1, 3, 4]


def _build_program():
    import concourse.bass as bass
    import concourse.tile as tile
    from concourse import bacc, mybir

    f32 = mybir.dt.float32
    f16 = mybir.dt.float16
    AF = mybir.ActivationFunctionType

    # frame-graph normalization (path graph + self loops): deg = 2 at ends, 3 inside
    deg_fr = np.full(F, 3.0, np.float32)
    deg_fr[0] = deg_fr[F - 1] = 2.0
    dinv = (1.0 / np.sqrt(deg_fr)).astype(np.float32)

    # Bacc (not plain Bass): its compile pipeline splits multi-waits into
    # event semaphores (TRN2 allows at most 1 sync wait per instruction).
    nc = bacc.Bacc(None, target_bir_lowering=False, debug=False)
    x_in = nc.declare_dram_parameter("x", [F, C, M], f16, isOutput=False)
    # A (5x512 cols) | Wi (2x256) | Wo (2x256) packed into one (128, 3584) input
    cst_in = nc.declare_dram_parameter("CST", [128, 5 * NLOC + 4 * C], f16, isOutput=False)
    out_d = nc.declare_dram_parameter("out", [F, C, NLOC], f32, isOutput=True)

    with tile.TileContext(nc) as tc:
        with (
            tc.tile_pool(name="consts", bufs=1) as cpool,
            tc.tile_pool(name="xin", bufs=3) as xpool,
            tc.tile_pool(name="psb", bufs=3) as ppool,
            tc.tile_pool(name="ysb", bufs=4) as ypool,
            tc.tile_pool(name="osb", bufs=2) as opool,
            tc.tile_pool(name="stmp", bufs=2) as spool,
            tc.tile_pool(name="pp", bufs=4, space="PSUM") as pp_ps,
            tc.tile_pool(name="pt", bufs=1, space="PSUM") as pt_ps,
            tc.tile_pool(name="pz", bufs=1, space="PSUM") as pz_ps,
        ):
            # ---- constants into SBUF (single DMA, sliced views) ----
            cst = cpool.tile([128, 5 * NLOC + 4 * C], f16, tag="CST")
            a_sb = [cst[:, 512 * j : 512 * (j + 1)] for j in range(5)]
            wi_sb = [cst[:, 2560 + 256 * kc : 2560 + 256 * (kc + 1)] for kc in range(2)]
            wo_sb = [cst[:, 3072 + 256 * kc : 3072 + 256 * (kc + 1)] for kc in range(2)]

            yh = {}       # f -> (128, 2, NLOC) fp16: dinv[f] * relu(T^T[f])
            opair = {}    # parity buffer for paired output DMA

            def emit_out(fp):
                # the frame stencil commutes with W_inter: first sum the (tiny
                # fp16) yh tiles on DVE, then one 4-matmul stage per frame
                terms = [t for t in (fp - 1, fp, fp + 1) if 0 <= t < F]
                ys = spool.tile([128, 2, NLOC], f16, tag="su", name="ys")
                for cp in range(2):
                    if len(terms) == 3:
                        t1 = spool.tile([128, NLOC], f16, tag=f"st{cp}", name=f"t1{cp}")
                        nc.vector.tensor_add(
                            t1[:], yh[terms[0]][:, cp, :], yh[terms[2]][:, cp, :]
                        )
                        nc.vector.tensor_add(
                            ys[:, cp, :], t1[:], yh[terms[1]][:, cp, :]
                        )
                    else:
                        nc.vector.tensor_add(
                            ys[:, cp, :], yh[terms[0]][:, cp, :], yh[terms[1]][:, cp, :]
                        )
                zs = pz_ps.tile([128, 2, NLOC], f32, tag="z")
                for co in range(2):
                    for kc in range(2):
                        nc.tensor.matmul(
                            zs[:, co, :],
                            wo_sb[kc][:, 128 * co : 128 * (co + 1)],
                            ys[:, kc, :],
                            start=(kc == 0),
                            stop=(kc == 1),
                        )
                # out = relu(dinv[fp] * Zs) straight from PSUM into the pair buffer
                par = fp % 2
                if par == 0:
                    opair[0] = opool.tile([128, 2, 2, NLOC], f32, tag="o", name="opair")
                o = opair[0]
                for co in range(2):
                    nc.scalar.activation(
                        o[:, par, co, :], zs[:, co, :], AF.Relu, scale=float(dinv[fp])
                    )
                if par == 1 or fp == F - 1:
                    f0 = fp - par
                    # DRAM view: out[f0:fp+1] as (p, f, ct, n) to match the tile
                    nc.sync.dma_start(
                        out=out_d[f0 : fp + 1, :, :].rearrange(
                            "f (ct p) n -> p f ct n", p=128
                        ),
                        in_=o[:, : par + 1, :, :],
                    )

            # x chunks with 2-chunk prefetch: loads stay ahead of compute
            starts = []
            _f0 = 0
            for w in _XCHUNKS:
                starts.append(_f0)
                _f0 += w
            chunk_of = {}
            for ci, (s0, w) in enumerate(zip(starts, _XCHUNKS)):
                for ff in range(s0, s0 + w):
                    chunk_of[ff] = ci
            xq_tiles = {}

            def load_chunk(ci):
                if ci >= len(_XCHUNKS) or ci in xq_tiles:
                    return
                s0, w = starts[ci], _XCHUNKS[ci]
                xq = xpool.tile([128, w, 2, M], f16, tag="x", name=f"xq{ci}")
                nc.sync.dma_start(
                    out=xq[:],
                    in_=x_in[s0 : s0 + w, :, :].rearrange(
                        "f (ct p) m -> p f ct m", p=128
                    ),
                )
                xq_tiles[ci] = xq

            # Wi/Wo (needed by the very first matmul) land before the A blocks
            nc.sync.dma_start(out=cst[:, 2560:], in_=cst_in[:, 2560:])
            load_chunk(0)
            nc.sync.dma_start(out=cst[:, :2560], in_=cst_in[:, :2560])
            load_chunk(1)
            load_chunk(2)
            for f in range(F):
                ci = chunk_of[f]
                if starts[ci] == f:
                    load_chunk(ci + 2)
                xq = xq_tiles[ci]
                fi = f - starts[ci]
                # ---- stage 1a: P (m part, c') ----
                p_sb = [None] * 5
                for mi in (2, 0, 1, 3, 4):
                    ps = pp_ps.tile([128, C], f32, tag="pp")
                    for kc in range(2):
                        nc.tensor.matmul(
                            ps[:],
                            xq[:, fi, kc, 128 * mi : 128 * (mi + 1)],
                            wi_sb[kc],
                            start=(kc == 0),
                            stop=(kc == 1),
                        )
                    sb = ppool.tile([128, C], f16, tag=f"p{mi}", name=f"p{mi}")
                    if mi == 2:
                        nc.scalar.copy(sb[:], ps[:])
                    else:
                        nc.vector.tensor_copy(sb[:], ps[:])
                    p_sb[mi] = sb
                # ---- stage 1c: T^T (c' part, n), banded accumulation ----
                ts = pt_ps.tile([128, 2, NLOC], f32, tag="t")
                for cp in range(2):
                    for oi, j in enumerate(_BAND_ORDER):
                        # group opener spans the full bank (A is zero outside
                        # its band) so later banded matmuls purely accumulate
                        n0, n1 = (0, NLOC) if oi == 0 else _BANDS[j]
                        nc.tensor.matmul(
                            ts[:, cp, n0:n1],
                            p_sb[j][:, 128 * cp : 128 * (cp + 1)],
                            a_sb[j][:, n0:n1],
                            start=(oi == 0),
                            stop=(oi == 4),
                            skip_group_check=True,
                        )
                yb = ypool.tile([128, 2, NLOC], f16, tag="y")
                for cp in range(2):
                    nc.scalar.activation(
                        yb[:, cp, :], ts[:, cp, :], AF.Relu, scale=float(dinv[f])
                    )
                yh[f] = yb
                if f >= 1:
                    emit_out(f - 1)
                    yh.pop(f - 2, None)
            emit_out(F - 1)

    # run the bacc compile pipeline (multi-wait splitting via event semaphores,
    # register allocation) — the axon SPMD exec path doesn't finalize for us
    nc.finalize()
    return nc


_CACHED = {}


def _get_program():
    if "nc" not in _CACHED:
        _CACHED["nc"] = _build_program()
    return _CACHED["nc"]


def build_in_maps(d_seq, W_intra, W_inter, adj_space, adj_frame=None):
    f16 = np.float16
    d_seq = np.asarray(d_seq, dtype=np.float32)
    W_intra = np.asarray(W_intra, dtype=np.float32)
    W_inter = np.asarray(W_inter, dtype=np.float32)
    adj_space = np.asarray(adj_space, dtype=np.float32)

    # host-side normalization of the spatial adjacency (tiny, deterministic)
    deg = adj_space.sum(-1)
    dinv_sp = 1.0 / np.sqrt(deg)
    A_sp = (adj_space * dinv_sp[:, None] * dinv_sp[None, :]).astype(f16)

    Wi16 = np.ascontiguousarray(W_intra.astype(f16))
    Wo16 = np.ascontiguousarray(W_inter.astype(f16))

    in_maps = []
    for core in range(NCORES):
        b, half = divmod(core, 2)
        own_lo = half * NLOC
        g_lo, g_hi = own_lo - HALO, own_lo + NLOC + HALO
        v_lo, v_hi = max(0, g_lo), min(N, g_hi)
        x_sl = np.zeros((F, C, M), dtype=f16)
        x_sl[:, :, v_lo - g_lo : v_hi - g_lo] = d_seq[b].reshape(F, C, N)[:, :, v_lo:v_hi]
        A_sl = np.zeros((M, NLOC), dtype=f16)
        A_sl[v_lo - g_lo : v_hi - g_lo, :] = A_sp[v_lo:v_hi, own_lo : own_lo + NLOC]
        cstm = np.zeros((128, 5 * NLOC + 4 * C), dtype=f16)
        for j in range(5):
            cstm[:, 512 * j : 512 * (j + 1)] = A_sl[128 * j : 128 * (j + 1), :]
        for kc in range(2):
            cstm[:, 2560 + 256 * kc : 2560 + 256 * (kc + 1)] = Wi16[128 * kc : 128 * (kc + 1), :]
            cstm[:, 3072 + 256 * kc : 3072 + 256 * (kc + 1)] = Wo16[128 * kc : 128 * (kc + 1), :]
        in_maps.append(
            {
                "x": np.ascontiguousarray(x_sl),
                "CST": cstm,
            }
        )

    return in_maps


def kernel(d_seq, W_intra, W_inter, adj_space, adj_frame):
    from concourse.bass_utils import run_bass_kernel_spmd

    d_seq = np.asarray(d_seq, dtype=np.float32)
    in_maps = build_in_maps(d_seq, W_intra, W_inter, adj_space, adj_frame)
    nc = _get_program()
    res = run_bass_kernel_spmd(nc, in_maps, list(range(NCORES)))

    out = np.zeros((BS, F, C, N), dtype=np.float32)
    for core in range(NCORES):
        b, half = divmod(core, 2)
        own_lo = half * NLOC
        out[b, :, :, own_lo : own_lo + NLOC] = res.results[core]["out"]
    return out.reshape(d_seq.shape)


# revision 27
# speedup vs baseline: 1.1582x; 1.1582x over previous
"""Trainium2 Bass kernel for nn_DifferentialGCNBlock (intra-spatial GCN + inter-frame GCN).

Sharding: 8 cores = 4 batches x 2 node-halves. Each core computes both GCN stages
fully locally for its (batch, 512-node half), using a 64-node halo on each side
(A_sp is a 3x3x3 stencil => bandwidth |i-j| <= 73 < 128).

Math per core (c-major activations matching DRAM layout, all matmul data fp16,
PSUM accumulation fp32):
  x^T[f] (C=256 part, M=640 free)  --DMA (fp16, 5 frames/transfer)-->  SBUF
  P[f]   = x W_intra          : lhsT = x^T slices (c,m), rhs = Wi  -> P (m part, c')
  T^T[f] = (A_sp P)^T         : lhsT = P slices (m,c'), rhs = A band blocks (m,n)
  yh[f]  = relu(dinv[f]*T^T)  : ACT, PSUM->SBUF fp16 (c' part, n)
  Zs[f'] = Wo^T (yh[f'-1]+yh[f']+yh[f'+1])   : 3-frame sum folded into PSUM
                                               accumulation (6 matmuls)
  out[f'] = relu(dinv[f']*Zs[f'])            : ACT, PSUM->SBUF fp32, paired DMA out
The inter-frame path graph's normalized adjacency is separable: A_fr[f',f] =
dinv[f']*dinv[f] for |f-f'|<=1, which is what the yh scaling + output scaling use.
"""
import sys

for p in ("/opt/trn_rl_repo",):
    if p not in sys.path:
        sys.path.insert(0, p)

import numpy as np

H, W_, D = 16, 8, 8
N = H * W_ * D          # 1024
F = 25
C = 256
BS = 4
HALO = 64
NLOC = 512
M = HALO + NLOC + HALO  # 640
NCORES = 8
FPL = 5                 # frames per input DMA (25 = 5*5)
_XCHUNKS = [1, 2, 3, 3, 4, 4, 4, 4]  # small first chunks so compute starts early

# banded structure of A_sp in local coords: m = n + 64 + delta, |delta| <= 73
# k-tile j (m in [128j, 128j+128)) touches n in [128j-137, 128j+137)
_BANDS = [(0, 144), (0, 272), (112, 400), (240, 512), (368, 512)]
# order: j=2's band [112,400) overlaps every other band, so putting it first
# (start=True, widened to the full bank) guarantees the whole-bank has_written
# clear happens first; the remaining matmuls accumulate per-element and are
# order-independent.
_BAND_ORDER = [2, 0, 1, 3, 4]


def _build_program():
    import concourse.bass as bass
    import concourse.tile as tile
    from concourse import bacc, mybir

    f32 = mybir.dt.float32
    f16 = mybir.dt.float16
    AF = mybir.ActivationFunctionType

    # frame-graph normalization (path graph + self loops): deg = 2 at ends, 3 inside
    deg_fr = np.full(F, 3.0, np.float32)
    deg_fr[0] = deg_fr[F - 1] = 2.0
    dinv = (1.0 / np.sqrt(deg_fr)).astype(np.float32)

    # Bacc (not plain Bass): its compile pipeline splits multi-waits into
    # event semaphores (TRN2 allows at most 1 sync wait per instruction).
    nc = bacc.Bacc(None, target_bir_lowering=False, debug=False)
    x_in = nc.declare_dram_parameter("x", [F, C, M], f16, isOutput=False)
    # A (5x512 cols) | Wi (2x256) | Wo (2x256) packed into one (128, 3584) input
    cst_in = nc.declare_dram_parameter("CST", [128, 5 * NLOC + 4 * C], f16, isOutput=False)
    out_d = nc.declare_dram_parameter("out", [F, C, NLOC], f32, isOutput=True)

    with tile.TileContext(nc) as tc:
        with (
            tc.tile_pool(name="consts", bufs=1) as cpool,
            tc.tile_pool(name="xin", bufs=3) as xpool,
            tc.tile_pool(name="psb", bufs=4) as ppool,
            tc.tile_pool(name="ysb", bufs=5) as ypool,
            tc.tile_pool(name="osb", bufs=3) as opool,
            tc.tile_pool(name="stmp", bufs=3) as spool,
            tc.tile_pool(name="pp", bufs=4, space="PSUM") as pp_ps,
            tc.tile_pool(name="pt", bufs=1, space="PSUM") as pt_ps,
            tc.tile_pool(name="pz", bufs=1, space="PSUM") as pz_ps,
        ):
            # ---- constants into SBUF (single DMA, sliced views) ----
            cst = cpool.tile([128, 5 * NLOC + 4 * C], f16, tag="CST")
            a_sb = [cst[:, 512 * j : 512 * (j + 1)] for j in range(5)]
            wi_sb = [cst[:, 2560 + 256 * kc : 2560 + 256 * (kc + 1)] for kc in range(2)]
            wo_sb = [cst[:, 3072 + 256 * kc : 3072 + 256 * (kc + 1)] for kc in range(2)]

            yh = {}       # f -> (128, 2, NLOC) fp16: dinv[f] * relu(T^T[f])
            opair = {}    # parity buffer for paired output DMA

            def emit_out(fp):
                # the frame stencil commutes with W_inter: first sum the (tiny
                # fp16) yh tiles on DVE, then one 4-matmul stage per frame
                terms = [t for t in (fp - 1, fp, fp + 1) if 0 <= t < F]
                if len(terms) == 3:
                    t1 = spool.tile([128, 2, NLOC], f16, tag="st")
                    nc.vector.tensor_add(t1[:], yh[terms[0]][:], yh[terms[2]][:])
                    ys = spool.tile([128, 2, NLOC], f16, tag="su")
                    nc.vector.tensor_add(ys[:], t1[:], yh[terms[1]][:])
                else:
                    ys = spool.tile([128, 2, NLOC], f16, tag="su")
                    nc.vector.tensor_add(ys[:], yh[terms[0]][:], yh[terms[1]][:])
                zs = pz_ps.tile([128, 2, NLOC], f32, tag="z")
                for co in range(2):
                    for kc in range(2):
                        nc.tensor.matmul(
                            zs[:, co, :],
                            wo_sb[kc][:, 128 * co : 128 * (co + 1)],
                            ys[:, kc, :],
                            start=(kc == 0),
                            stop=(kc == 1),
                        )
                # out = relu(dinv[fp] * Zs) straight from PSUM into the pair buffer
                par = fp % 2
                if par == 0:
                    opair[0] = opool.tile([128, 2, 2, NLOC], f32, tag="o", name="opair")
                o = opair[0]
                nc.scalar.activation(
                    o[:, par, :, :], zs[:], AF.Relu, scale=float(dinv[fp])
                )
                if par == 1 or fp == F - 1:
                    f0 = fp - par
                    # DRAM view: out[f0:fp+1] as (p, f, ct, n) to match the tile
                    nc.sync.dma_start(
                        out=out_d[f0 : fp + 1, :, :].rearrange(
                            "f (ct p) n -> p f ct n", p=128
                        ),
                        in_=o[:, : par + 1, :, :],
                    )

            # x chunks with 2-chunk prefetch: loads stay ahead of compute
            starts = []
            _f0 = 0
            for w in _XCHUNKS:
                starts.append(_f0)
                _f0 += w
            chunk_of = {}
            for ci, (s0, w) in enumerate(zip(starts, _XCHUNKS)):
                for ff in range(s0, s0 + w):
                    chunk_of[ff] = ci
            xq_tiles = {}

            def load_chunk(ci):
                if ci >= len(_XCHUNKS) or ci in xq_tiles:
                    return
                s0, w = starts[ci], _XCHUNKS[ci]
                xq = xpool.tile([128, w, 2, M], f16, tag="x", name=f"xq{ci}")
                nc.sync.dma_start(
                    out=xq[:],
                    in_=x_in[s0 : s0 + w, :, :].rearrange(
                        "f (ct p) m -> p f ct m", p=128
                    ),
                )
                xq_tiles[ci] = xq

            # Wi/Wo (needed by the very first matmul) land before the A blocks
            nc.sync.dma_start(out=cst[:, 2560:], in_=cst_in[:, 2560:])
            load_chunk(0)
            nc.sync.dma_start(out=cst[:, :2560], in_=cst_in[:, :2560])
            load_chunk(1)
            load_chunk(2)
            for f in range(F):
                ci = chunk_of[f]
                if starts[ci] == f:
                    load_chunk(ci + 2)
                xq = xq_tiles[ci]
                fi = f - starts[ci]
                # ---- stage 1a: P (m part, c') ----
                p_sb = [None] * 5
                for mi in (2, 0, 1, 3, 4):
                    ps = pp_ps.tile([128, C], f32, tag="pp")
                    for kc in range(2):
                        nc.tensor.matmul(
                            ps[:],
                            xq[:, fi, kc, 128 * mi : 128 * (mi + 1)],
                            wi_sb[kc],
                            start=(kc == 0),
                            stop=(kc == 1),
                        )
                    sb = ppool.tile([128, C], f16, tag=f"p{mi}", name=f"p{mi}")
                    if mi == 2:
                        nc.scalar.copy(sb[:], ps[:])
                    else:
                        nc.vector.tensor_copy(sb[:], ps[:])
                    p_sb[mi] = sb
                # ---- stage 1c: T^T (c' part, n), banded accumulation ----
                ts = pt_ps.tile([128, 2, NLOC], f32, tag="t")
                for cp in range(2):
                    for oi, j in enumerate(_BAND_ORDER):
                        # group opener spans the full bank (A is zero outside
                        # its band) so later banded matmuls purely accumulate
                        n0, n1 = (0, NLOC) if oi == 0 else _BANDS[j]
                        nc.tensor.matmul(
                            ts[:, cp, n0:n1],
                            p_sb[j][:, 128 * cp : 128 * (cp + 1)],
                            a_sb[j][:, n0:n1],
                            start=(oi == 0),
                            stop=(oi == 4),
                            skip_group_check=True,
                        )
                yb = ypool.tile([128, 2, NLOC], f16, tag="y")
                nc.scalar.activation(yb[:], ts[:], AF.Relu, scale=float(dinv[f]))
                yh[f] = yb
                if f >= 1:
                    emit_out(f - 1)
                    yh.pop(f - 2, None)
            emit_out(F - 1)

    # run the bacc compile pipeline (multi-wait splitting via event semaphores,
    # register allocation) — the axon SPMD exec path doesn't finalize for us
    nc.finalize()
    return nc


_CACHED = {}


def _get_program():
    if "nc" not in _CACHED:
        _CACHED["nc"] = _build_program()
    return _CACHED["nc"]


def build_in_maps(d_seq, W_intra, W_inter, adj_space, adj_frame=None):
    f16 = np.float16
    d_seq = np.asarray(d_seq, dtype=np.float32)
    W_intra = np.asarray(W_intra, dtype=np.float32)
    W_inter = np.asarray(W_inter, dtype=np.float32)
    adj_space = np.asarray(adj_space, dtype=np.float32)

    # host-side normalization of the spatial adjacency (tiny, deterministic)
    deg = adj_space.sum(-1)
    dinv_sp = 1.0 / np.sqrt(deg)
    A_sp = (adj_space * dinv_sp[:, None] * dinv_sp[None, :]).astype(f16)

    Wi16 = np.ascontiguousarray(W_intra.astype(f16))
    Wo16 = np.ascontiguousarray(W_inter.astype(f16))

    in_maps = []
    for core in range(NCORES):
        b, half = divmod(core, 2)
        own_lo = half * NLOC
        g_lo, g_hi = own_lo - HALO, own_lo + NLOC + HALO
        v_lo, v_hi = max(0, g_lo), min(N, g_hi)
        x_sl = np.zeros((F, C, M), dtype=f16)
        x_sl[:, :, v_lo - g_lo : v_hi - g_lo] = d_seq[b].reshape(F, C, N)[:, :, v_lo:v_hi]
        A_sl = np.zeros((M, NLOC), dtype=f16)
        A_sl[v_lo - g_lo : v_hi - g_lo, :] = A_sp[v_lo:v_hi, own_lo : own_lo + NLOC]
        cstm = np.zeros((128, 5 * NLOC + 4 * C), dtype=f16)
        for j in range(5):
            cstm[:, 512 * j : 512 * (j + 1)] = A_sl[128 * j : 128 * (j + 1), :]
        for kc in range(2):
            cstm[:, 2560 + 256 * kc : 2560 + 256 * (kc + 1)] = Wi16[128 * kc : 128 * (kc + 1), :]
            cstm[:, 3072 + 256 * kc : 3072 + 256 * (kc + 1)] = Wo16[128 * kc : 128 * (kc + 1), :]
        in_maps.append(
            {
                "x": np.ascontiguousarray(x_sl),
                "CST": cstm,
            }
        )

    return in_maps


def kernel(d_seq, W_intra, W_inter, adj_space, adj_frame):
    from concourse.bass_utils import run_bass_kernel_spmd

    d_seq = np.asarray(d_seq, dtype=np.float32)
    in_maps = build_in_maps(d_seq, W_intra, W_inter, adj_space, adj_frame)
    nc = _get_program()
    res = run_bass_kernel_spmd(nc, in_maps, list(range(NCORES)))

    out = np.zeros((BS, F, C, N), dtype=np.float32)
    for core in range(NCORES):
        b, half = divmod(core, 2)
        own_lo = half * NLOC
        out[b, :, :, own_lo : own_lo + NLOC] = res.results[core]["out"]
    return out.reshape(d_seq.shape)
